# revision 5
# baseline (speedup 1.0000x reference)
"""Trainium2 Bass kernel for Chn8ActGrp3WgtQuantizedLinear — fp8 DoubleRow version.

Computes: out = fake_quant8_per_row(x) @ dequant(weight_qvals, weight_scales).T

  x:             (1024, 4096)  f32
  weight_qvals:  (11008, 4096) int32, 3-bit values in [-4, 3]
  weight_scales: (11008, 32)   f32
  out:           (1024, 11008) f32

Strategy (tensor parallel over 8 NeuronCores, shard N -> 1376/core):
  Host:
    W = dequant(qvals, scales); c = max|W|; Wn = W/c
    Wh = e4m3(Wn); Wl = e4m3(Wn - Wh)      (two-term fp8 representation)
    ship K-major fp8: wh16 = 16*Wh (exact in e4m3), wl16 = 16*Wl (exact)
    corr rows (fp16 hi/lo): -colsum(Wh+Wl) and 7.5*colsum(Wl); x -> fp16
  Device, per m-tile (128 rows):
    stats (min/max) -> scale/inv/zero; u = x*inv + (MAGIC+zero) (rounds RNE)
    qxu = int16(u - MAGIC + 128) in [0,255] -> batched XBAR transpose [k,m]
    DVE split + ACT cast (fp8, all values e4m3-exact):
      q1u  = qxu >> 4 (logical)  in [0, 15]
      q0s  = (qxu & 15) * 1/16   in {0, 1/16, ..., 15/16}
    (+128 bias folds into the correction row: (zero+128) * -colsum)
    fp8 DoubleRow matmuls (0.5 cyc/row), three pair-sweeps over one weight set:
      psum += q1f_p @ wh16_p     (pairs p)   [= 16*q1f @ Wh]
      psum += q0s_p @ wh16_p                 [= q0u @ Wh]
      psum += q1f_p @ wl16_p                 [= 16*q1f @ Wl]
      => psum = qx @ (Wh+Wl) - q0u @ Wl  (last term dropped, mean-corrected)
    K=4 fp16 corr matmul into psum: zero_m*(-colsum(Wh+Wl)) + 7.5*colsum(Wl)
    evict: out = psum * scale_m  (ACT, fp16), host multiplies by c.
"""

import sys
import types

import numpy as np
import ml_dtypes

M, K, N, GS = 1024, 4096, 11008, 128
NCORES = 8
NC = N // NCORES  # 1376
NGRP = K // GS  # 32
NPAIR = NGRP // 2
MTILES = M // 128  # 8
XCH = 1024  # x/u/qxi processing chunk along K
NXC = K // XCH
MAGIC = 12582912.0  # 1.5 * 2**23
E4 = ml_dtypes.float8_e4m3

_CACHE = {}
LAST_RESULTS = None


def _install_axon_ntff_hook():
    try:
        if "antenv.axon_hooks" in sys.modules:
            return
        import antenv

        mod = types.ModuleType("antenv.axon_hooks")
        _state = {"hook": None}
        mod.set_axon_ntff_profile_hook = lambda h: _state.__setitem__("hook", h)
        mod.get_axon_ntff_profile_hook = lambda: _state["hook"]
        sys.modules["antenv.axon_hooks"] = mod
        antenv.axon_hooks = mod

        from trn_agent_boot.trn_boot import _ntff_profile_via_ctypes

        mod.set_axon_ntff_profile_hook(
            _ntff_profile_via_ctypes("/opt/axon/libaxon_pjrt.so")
        )
    except Exception:
        pass


def _build():
    if "nc" in _CACHE:
        return _CACHE["nc"]

    import concourse.bass as bass
    import concourse.tile as tile
    from concourse import bacc, mybir
    from concourse.masks import make_identity

    dt = mybir.dt
    F32, F16, F8, I16 = dt.float32, dt.float16, dt.float8e4, dt.int16
    ALU = mybir.AluOpType
    ACTF = mybir.ActivationFunctionType
    AX = mybir.AxisListType
    DR = mybir.MatmulPerfMode.DoubleRow

    nc = bacc.Bacc("TRN2", target_bir_lowering=False, debug=False,
                   num_devices=NCORES)

    x_d = nc.dram_tensor("x", [M, K], F16, kind="ExternalInput").ap()
    wh16_d = nc.dram_tensor("wh16", [K, NC], F8, kind="ExternalInput").ap()
    wl16_d = nc.dram_tensor("wl16", [K, NC], F8, kind="ExternalInput").ap()
    crow_d = nc.dram_tensor("crow", [4, NC], F16, kind="ExternalInput").ap()
    out_d = nc.dram_tensor("out", [M, NC], F16, kind="ExternalOutput").ap()

    CHUNKS = [(c, min(512, NC - c)) for c in range(0, NC, 512)]

    with tile.TileContext(nc) as tc:
        import contextlib

        ctx = contextlib.ExitStack()
        with ctx:
            consts = ctx.enter_context(tc.tile_pool(name="consts", bufs=1))
            whpool = ctx.enter_context(tc.tile_pool(name="wh", bufs=1))
            wlpool = ctx.enter_context(tc.tile_pool(name="wl", bufs=1))
            xp = ctx.enter_context(tc.tile_pool(name="x", bufs=3))
            up = ctx.enter_context(tc.tile_pool(name="u", bufs=2))
            qip = ctx.enter_context(tc.tile_pool(name="qi", bufs=2))
            qtp = ctx.enter_context(tc.tile_pool(name="qt", bufs=2))
            lhp = ctx.enter_context(tc.tile_pool(name="lh", bufs=2))
            qop = ctx.enter_context(tc.tile_pool(name="q0", bufs=2))
            outp = ctx.enter_context(tc.tile_pool(name="o", bufs=2))
            vecs = ctx.enter_context(tc.tile_pool(name="v", bufs=2))
            ps_out = ctx.enter_context(
                tc.tile_pool(name="pso", bufs=2, space="PSUM"))
            ps_c = ctx.enter_context(
                tc.tile_pool(name="psc", bufs=1, space="PSUM"))

            ident = consts.tile([128, 128], F16)
            make_identity(nc, ident[:])
            neg_magic_vec = consts.tile([128, 1], F32)
            nc.vector.memset(neg_magic_vec[:], -(MAGIC - 128.0))

            # weights: [128, 32(g), NC] fp8, single copies
            wh = whpool.tile([128, NGRP, NC], F8)
            wl = wlpool.tile([128, NGRP, NC], F8)
            crow = consts.tile([4, NC], F16)
            nc.scalar.dma_start(crow[:], crow_d[:, :])
            for g in range(NGRP):
                sl = slice(g * 128, (g + 1) * 128)
                nc.scalar.dma_start(wh[:, g, :], wh16_d[sl, :])
            for g in range(NGRP):
                sl = slice(g * 128, (g + 1) * 128)
                nc.scalar.dma_start(wl[:, g, :], wl16_d[sl, :])

            scp_of = {}
            lrow_of = {}
            lhs_of = {}

            def quant_phase(m):
                x_t = xp.tile([128, K], F16, tag="xt")
                mxp = vecs.tile([128, NXC], F16, tag="mxp")
                mnp = vecs.tile([128, NXC], F16, tag="mnp")
                for j in range(NXC):
                    sl = slice(j * XCH, (j + 1) * XCH)
                    nc.sync.dma_start(x_t[:, sl], x_d[m * 128:(m + 1) * 128, sl])
                    nc.vector.tensor_reduce(mxp[:, j:j + 1], x_t[:, sl],
                                            axis=AX.X, op=ALU.max)
                    nc.vector.tensor_reduce(mnp[:, j:j + 1], x_t[:, sl],
                                            axis=AX.X, op=ALU.min)
                mx = vecs.tile([128, 1], F32, tag="mx")
                nc.vector.tensor_reduce(mx[:], mxp[:], axis=AX.X, op=ALU.max)
                mn = vecs.tile([128, 1], F32, tag="mn")
                nc.vector.tensor_reduce(mn[:], mnp[:], axis=AX.X, op=ALU.min)
                xc = vecs.tile([128, 1], F32, tag="xc")
                nc.vector.tensor_scalar(xc[:], mx[:], 0.0, None, ALU.max)
                nn_ = vecs.tile([128, 1], F32, tag="nn")
                nc.vector.tensor_scalar(nn_[:], mn[:], 0.0, None, ALU.min)
                df = vecs.tile([128, 1], F32, tag="df")
                nc.vector.tensor_tensor(df[:], xc[:], nn_[:], ALU.subtract)
                sc = vecs.tile([128, 1], F32, tag="sc")
                nc.vector.tensor_scalar(sc[:], df[:], 1.0 / 255.0, 1e-9,
                                        ALU.mult, ALU.max)
                inv = vecs.tile([128, 1], F32, tag="inv")
                nc.vector.reciprocal(inv[:], sc[:])
                # zero = round(-128 - nn*inv); zr = MAGIC + zero
                z0 = vecs.tile([128, 1], F32, tag="z0")
                nc.vector.tensor_tensor(z0[:], nn_[:], inv[:], ALU.mult)
                z1 = vecs.tile([128, 1], F32, tag="z1")
                nc.vector.tensor_scalar(z1[:], z0[:], -1.0, -128.0,
                                        ALU.mult, ALU.add)
                zr = vecs.tile([128, 1], F32, tag="zr")
                nc.vector.tensor_scalar(zr[:], z1[:], MAGIC, None, ALU.add)

                # u -> int16 qxi -> XBAR transpose, chunked along K
                qxt = qtp.tile([128, NGRP, 128], I16, tag="qxt")
                gj = XCH // 128
                for j in range(NXC):
                    sl = slice(j * XCH, (j + 1) * XCH)
                    u = up.tile([128, XCH], F32, tag="u")
                    nc.vector.tensor_scalar(u[:], x_t[:, sl], inv[:], zr[:],
                                            ALU.mult, ALU.add)
                    qxi = qip.tile([128, XCH], I16, tag="qxi")
                    nc.scalar.activation(qxi[:], u[:], ACTF.Identity,
                                         bias=neg_magic_vec[:], scale=1.0)
                    nc.sync.dma_start(qxt[:, j * gj:(j + 1) * gj, :], qxi[:],
                                      transpose=True)

                # splits -> fp8 lhs [128, 2(which), 32(g), 128(m)]
                # bit-ops can't cast (verifier): DVE shifts/masks keep int16,
                # ACT casts to fp8 (x1/16 folded into the ACT scale for q0)
                lhs = lhp.tile([128, 2, NGRP, 128], F8, tag="lhs")
                tq1 = qop.tile([128, NGRP, 128], I16, tag="tq1")
                nc.vector.tensor_scalar(tq1[:], qxt[:], 4, None,
                                        ALU.logical_shift_right)
                nc.scalar.activation(lhs[:, 0, :, :], tq1[:], ACTF.Identity,
                                     bias=0.0, scale=1.0)
                tq0 = qop.tile([128, NGRP, 128], I16, tag="tq0")
                nc.vector.tensor_scalar(tq0[:], qxt[:], 15, None,
                                        ALU.bitwise_and)
                nc.scalar.activation(lhs[:, 1, :, :], tq0[:], ACTF.Identity,
                                     bias=0.0, scale=0.0625)

                # corr lhsT: columns [zero, zero, 1, 1] -> transpose to [4,128]
                csrc = vecs.tile([128, 4], F16, tag="csrc")
                nc.vector.tensor_scalar(csrc[:, 0:1], zr[:],
                                        -(MAGIC - 128.0), None, ALU.add)
                nc.vector.tensor_scalar(csrc[:, 1:2], zr[:],
                                        -(MAGIC - 128.0), None, ALU.add)
                nc.vector.memset(csrc[:, 2:4], 1.0)
                pc = ps_c.tile([4, 128], F16, tag="pc")
                nc.tensor.transpose(pc[:], csrc[:], ident[:])
                lrow = vecs.tile([4, 128], F16, tag="lrow")
                nc.scalar.copy(lrow[:], pc[:])

                scp_of[m] = sc
                lrow_of[m] = lrow
                lhs_of[m] = lhs

            def mm_phase(m):
                lhs = lhs_of[m]
                psum = ps_out.tile([128, NC], F32, tag="psum")
                for (c0, cw) in CHUNKS:
                    nc.tensor.matmul(psum[:, c0:c0 + cw],
                                     lhsT=lrow_of[m][:, :],
                                     rhs=crow[:, c0:c0 + cw],
                                     start=True, stop=False)
                for which, wtile, is_last in ((0, wh, False), (1, wh, False),
                                              (0, wl, True)):
                    for p in range(NPAIR):
                        stop = is_last and (p == NPAIR - 1)
                        for (c0, cw) in CHUNKS:
                            nc.tensor.matmul(
                                psum[:, c0:c0 + cw],
                                lhsT=lhs[:, which, 2 * p:2 * p + 2, :],
                                rhs=wtile[:, 2 * p:2 * p + 2, c0:c0 + cw],
                                start=False, stop=stop, perf_mode=DR)
                return psum

            def evict_phase(m, psum):
                o_t = outp.tile([128, NC], F16, tag="o")
                nc.scalar.activation(o_t[:], psum[:], ACTF.Identity,
                                     bias=0.0, scale=scp_of[m][:])
                nc.scalar.dma_start(out_d[m * 128:(m + 1) * 128, :], o_t[:])

            quant_phase(0)
            quant_phase(1)
            for m in range(MTILES):
                psum = mm_phase(m)
                evict_phase(m, psum)
                if m + 2 < MTILES:
                    quant_phase(m + 2)

    nc.compile()
    _CACHE["nc"] = nc
    return nc


def _host_pack(weight_qvals, weight_scales):
    """Dequant + fp8 two-term split + per-core K-major packing."""
    wq = np.asarray(weight_qvals).astype(np.float32)
    ws = np.asarray(weight_scales, dtype=np.float32)
    Wf = (wq.reshape(N, NGRP, GS) * ws[:, :, None]).reshape(N, K)
    del wq
    c = float(np.abs(Wf).max())
    Wn = Wf / np.float32(c)
    del Wf
    Wh8 = Wn.astype(E4)
    Whf = Wh8.astype(np.float32)
    Wl8 = (Wn - Whf).astype(E4)
    Wlf = Wl8.astype(np.float32)
    del Wn
    wh16 = (Whf * np.float32(16.0)).astype(E4)   # exact
    wl16 = (Wlf * np.float32(16.0)).astype(E4)   # exact
    del Wh8, Wl8
    cs = -(Whf.astype(np.float64) + Wlf.astype(np.float64)).sum(axis=1)
    mm = 7.5 * Wlf.astype(np.float64).sum(axis=1)
    del Whf, Wlf
    cs_hi = cs.astype(np.float16)
    cs_lo = (cs - cs_hi.astype(np.float64)).astype(np.float16)
    m_hi = mm.astype(np.float16)
    m_lo = (mm - m_hi.astype(np.float64)).astype(np.float16)

    shards = []
    for ci in range(NCORES):
        sl = slice(ci * NC, (ci + 1) * NC)
        crow = np.stack([cs_hi[sl], cs_lo[sl], m_hi[sl], m_lo[sl]]).astype(
            np.float16)
        shards.append({
            "wh16": np.ascontiguousarray(wh16[sl].T),
            "wl16": np.ascontiguousarray(wl16[sl].T),
            "crow": crow,
        })
    return shards, c


def kernel(x, weight_qvals, weight_scales, group_size):
    global LAST_RESULTS
    _install_axon_ntff_hook()
    from concourse.bass_utils import run_bass_kernel_spmd

    x = np.asarray(x, dtype=np.float32)
    assert int(group_size) == GS
    assert x.shape == (M, K)

    nc = _build()
    shards, c = _host_pack(weight_qvals, weight_scales)
    x16 = x.astype(np.float16)

    in_maps = []
    for ci in range(NCORES):
        d = {"x": x16}
        d.update(shards[ci])
        in_maps.append(d)

    res = run_bass_kernel_spmd(nc, in_maps, core_ids=list(range(NCORES)))
    LAST_RESULTS = res
    out = np.concatenate(
        [r["out"].astype(np.float32) for r in res.results], axis=1)
    out *= np.float32(c)
    return out


if __name__ == "__main__":
    rng = np.random.default_rng(0)
    xv = rng.standard_normal((M, K)).astype(np.float32)
    wqv = rng.integers(-4, 4, (N, K)).astype(np.int32)
    wsv = (rng.random((N, NGRP)).astype(np.float32) * 0.02 + 1e-4)
    o = kernel(xv, wqv, wsv, GS)
    print("out shape:", o.shape, "finite:", np.isfinite(o).all())


# revision 6
# speedup vs baseline: 1.5715x; 1.5715x over previous
"""Trainium2 Bass kernel for Chn8ActGrp3WgtQuantizedLinear — fp16 XBAR version.

Computes: out = fake_quant8_per_row(x) @ dequant(weight_qvals, weight_scales).T

  x:             (1024, 4096)  f32
  weight_qvals:  (11008, 4096) int32, 3-bit values in [-4, 3]
  weight_scales: (11008, 32)   f32
  out:           (1024, 11008) f32

Strategy (tensor parallel over 8 NeuronCores, shard N -> 1376/core):
  Host: W = dequant(qvals, scales) -> fp16, K-major per core; x -> fp16.
  Device, per m-tile (128 rows):
    stats (min/max, fp16 2x DVE) -> scale sc, inv, zr = MAGIC + zero
    u = x*inv + zr           (one DVE ts; the add rounds RNE to MAGIC + qx)
    a = fp16(u - zr)         (ACT; = qx - zero, integer in [-255,255], exact)
    aT via batched DMA-XBAR transpose (16x128 tiles, no PE involvement)
    psum[m,n] += aT_g.T @ W_g  over 32 k-groups (fp16 matmuls, hidden ldw)
    evict: out = psum * sc  (ACT, fp16 out)
  PE queue is pure matmuls: quant work lives on DVE/ACT/DMA rings.
  Rings: sync = x loads + XBAR transposes; scalar = weights + out stores.
"""

import sys
import types

import numpy as np

M, K, N, GS = 1024, 4096, 11008, 128
NCORES = 8
NC = N // NCORES  # 1376
NGRP = K // GS  # 32
MTILES = M // 128  # 8
XCH = 1024
NXC = K // XCH
MAGIC = 12582912.0  # 1.5 * 2**23

_CACHE = {}
LAST_RESULTS = None


def _install_axon_ntff_hook():
    try:
        if "antenv.axon_hooks" in sys.modules:
            return
        import antenv

        mod = types.ModuleType("antenv.axon_hooks")
        _state = {"hook": None}
        mod.set_axon_ntff_profile_hook = lambda h: _state.__setitem__("hook", h)
        mod.get_axon_ntff_profile_hook = lambda: _state["hook"]
        sys.modules["antenv.axon_hooks"] = mod
        antenv.axon_hooks = mod

        from trn_agent_boot.trn_boot import _ntff_profile_via_ctypes

        mod.set_axon_ntff_profile_hook(
            _ntff_profile_via_ctypes("/opt/axon/libaxon_pjrt.so")
        )
    except Exception:
        pass


def _build():
    if "nc" in _CACHE:
        return _CACHE["nc"]

    import concourse.bass as bass
    import concourse.tile as tile
    from concourse import bacc, mybir

    dt = mybir.dt
    F32, F16 = dt.float32, dt.float16
    ALU = mybir.AluOpType
    ACTF = mybir.ActivationFunctionType
    AX = mybir.AxisListType

    nc = bacc.Bacc("TRN2", target_bir_lowering=False, debug=False,
                   num_devices=NCORES)

    x_d = nc.dram_tensor("x", [M, K], F16, kind="ExternalInput").ap()
    w_d = nc.dram_tensor("w16", [K, NC], F16, kind="ExternalInput").ap()
    out_d = nc.dram_tensor("out", [M, NC], F16, kind="ExternalOutput").ap()

    CHUNKS = [(c, min(512, NC - c)) for c in range(0, NC, 512)]

    with tile.TileContext(nc) as tc:
        import contextlib

        ctx = contextlib.ExitStack()
        with ctx:
            whpool = ctx.enter_context(tc.tile_pool(name="wh", bufs=1))
            xp = ctx.enter_context(tc.tile_pool(name="x", bufs=3))
            up = ctx.enter_context(tc.tile_pool(name="u", bufs=2))
            ap_ = ctx.enter_context(tc.tile_pool(name="a", bufs=2))
            atp = ctx.enter_context(tc.tile_pool(name="at", bufs=2))
            outp = ctx.enter_context(tc.tile_pool(name="o", bufs=2))
            vecs = ctx.enter_context(tc.tile_pool(name="v", bufs=2))
            ps_out = ctx.enter_context(
                tc.tile_pool(name="pso", bufs=2, space="PSUM"))

            # weights [128, 32(g), NC] fp16, streamed group by group
            w = whpool.tile([128, NGRP, NC], F16)
            for g in range(NGRP):
                nc.scalar.dma_start(w[:, g, :], w_d[g * 128:(g + 1) * 128, :])

            scp_of = {}
            at_of = {}

            def quant_phase(m):
                x_t = xp.tile([128, K], F16, tag="xt")
                mxp = vecs.tile([128, NXC], F16, tag="mxp")
                mnp = vecs.tile([128, NXC], F16, tag="mnp")
                for j in range(NXC):
                    sl = slice(j * XCH, (j + 1) * XCH)
                    nc.sync.dma_start(x_t[:, sl], x_d[m * 128:(m + 1) * 128, sl])
                    nc.vector.tensor_reduce(mxp[:, j:j + 1], x_t[:, sl],
                                            axis=AX.X, op=ALU.max)
                    nc.vector.tensor_reduce(mnp[:, j:j + 1], x_t[:, sl],
                                            axis=AX.X, op=ALU.min)
                mx = vecs.tile([128, 1], F32, tag="mx")
                nc.vector.tensor_reduce(mx[:], mxp[:], axis=AX.X, op=ALU.max)
                mn = vecs.tile([128, 1], F32, tag="mn")
                nc.vector.tensor_reduce(mn[:], mnp[:], axis=AX.X, op=ALU.min)
                xc = vecs.tile([128, 1], F32, tag="xc")
                nc.vector.tensor_scalar(xc[:], mx[:], 0.0, None, ALU.max)
                nn_ = vecs.tile([128, 1], F32, tag="nn")
                nc.vector.tensor_scalar(nn_[:], mn[:], 0.0, None, ALU.min)
                df = vecs.tile([128, 1], F32, tag="df")
                nc.vector.tensor_tensor(df[:], xc[:], nn_[:], ALU.subtract)
                sc = vecs.tile([128, 1], F32, tag="sc")
                nc.vector.tensor_scalar(sc[:], df[:], 1.0 / 255.0, 1e-9,
                                        ALU.mult, ALU.max)
                inv = vecs.tile([128, 1], F32, tag="inv")
                nc.vector.reciprocal(inv[:], sc[:])
                z0 = vecs.tile([128, 1], F32, tag="z0")
                nc.vector.tensor_tensor(z0[:], nn_[:], inv[:], ALU.mult)
                z1 = vecs.tile([128, 1], F32, tag="z1")
                nc.vector.tensor_scalar(z1[:], z0[:], -1.0, -128.0,
                                        ALU.mult, ALU.add)
                zr = vecs.tile([128, 1], F32, tag="zr")
                nc.vector.tensor_scalar(zr[:], z1[:], MAGIC, None, ALU.add)
                nzr = vecs.tile([128, 1], F32, tag="nzr")
                nc.vector.tensor_scalar(nzr[:], zr[:], -1.0, None, ALU.mult)

                aT = atp.tile([128, NGRP, 128], F16, tag="aT")
                gj = XCH // 128
                for j in range(NXC):
                    sl = slice(j * XCH, (j + 1) * XCH)
                    u = up.tile([128, XCH], F32, tag="u")
                    nc.vector.tensor_scalar(u[:], x_t[:, sl], inv[:], zr[:],
                                            ALU.mult, ALU.add)
                    a_t = ap_.tile([128, XCH], F16, tag="a")
                    nc.scalar.activation(a_t[:], u[:], ACTF.Identity,
                                         bias=nzr[:], scale=1.0)
                    nc.sync.dma_start(aT[:, j * gj:(j + 1) * gj, :], a_t[:],
                                      transpose=True)
                scp_of[m] = sc
                at_of[m] = aT

            def mm_phase(m):
                aT = at_of[m]
                psum = ps_out.tile([128, NC], F32, tag="psum")
                for g in range(NGRP):
                    for (c0, cw) in CHUNKS:
                        nc.tensor.matmul(psum[:, c0:c0 + cw],
                                         lhsT=aT[:, g, :],
                                         rhs=w[:, g, c0:c0 + cw],
                                         start=(g == 0), stop=(g == NGRP - 1))
                return psum

            def evict_phase(m, psum):
                o_t = outp.tile([128, NC], F16, tag="o")
                nc.scalar.activation(o_t[:], psum[:], ACTF.Identity,
                                     bias=0.0, scale=scp_of[m][:])
                nc.scalar.dma_start(out_d[m * 128:(m + 1) * 128, :], o_t[:])

            quant_phase(0)
            quant_phase(1)
            for m in range(MTILES):
                psum = mm_phase(m)
                evict_phase(m, psum)
                if m + 2 < MTILES:
                    quant_phase(m + 2)

    nc.compile()
    _CACHE["nc"] = nc
    return nc


def _host_pack(weight_qvals, weight_scales):
    wq = np.asarray(weight_qvals).astype(np.float32)
    ws = np.asarray(weight_scales, dtype=np.float32)
    Wf = (wq.reshape(N, NGRP, GS) * ws[:, :, None]).reshape(N, K)
    w16 = Wf.astype(np.float16)
    del Wf, wq
    shards = []
    for ci in range(NCORES):
        sl = slice(ci * NC, (ci + 1) * NC)
        shards.append({"w16": np.ascontiguousarray(w16[sl].T)})
    return shards


def kernel(x, weight_qvals, weight_scales, group_size):
    global LAST_RESULTS
    _install_axon_ntff_hook()
    from concourse.bass_utils import run_bass_kernel_spmd

    x = np.asarray(x, dtype=np.float32)
    assert int(group_size) == GS
    assert x.shape == (M, K)

    nc = _build()
    shards = _host_pack(weight_qvals, weight_scales)
    x16 = x.astype(np.float16)

    in_maps = []
    for ci in range(NCORES):
        d = {"x": x16}
        d.update(shards[ci])
        in_maps.append(d)

    res = run_bass_kernel_spmd(nc, in_maps, core_ids=list(range(NCORES)))
    LAST_RESULTS = res
    out = np.concatenate(
        [r["out"].astype(np.float32) for r in res.results], axis=1)
    return out


if __name__ == "__main__":
    rng = np.random.default_rng(0)
    xv = rng.standard_normal((M, K)).astype(np.float32)
    wqv = rng.integers(-4, 4, (N, K)).astype(np.int32)
    wsv = (rng.random((N, NGRP)).astype(np.float32) * 0.02 + 1e-4)
    o = kernel(xv, wqv, wsv, GS)
    print("out shape:", o.shape, "finite:", np.isfinite(o).all())
